# revision 25
# baseline (speedup 1.0000x reference)
"""Multi-head attention (B=4, S=2048, D=512, H=8) on 8 trn2 NeuronCores.

Sharding: core c = (batch b = c//2, query-half qh = c%2). Each core computes
the full attention output for 1024 query rows of one batch element.

Device-side scheme:
  - K/Q/V projections in bf16 at full PE rate (FWL weight loads); kT/qT/v
    drained to SBUF as float32r so logits and AV matmuls run on the
    full-rate fp32 PE path with ~fp22 accuracy (f32r costs the same PE
    streaming time as bf16 at N=512, and the same LDWEIGHTS time for these
    shapes since K=64 / M=65 never qualify for fast-weight-load anyway).
  - Logits transposed (lg^T[s_k, q]), two heads per pass as concurrent
    K=64 row-tiles (partitions 0-63 / 64-127).
  - exp on ACT (f32 PSUM -> f32r SBUF) with a per-key-partition mask bias;
    no max-subtraction (logits*scale ~ N(0,1); exp is safe in fp32) and
    masked/padded keys get bias -1e9 -> exp exactly 0.
  - AV: per key tile, lhsT = [v_h | ones] (M=65) so PSUM gets the attention
    numerator on partitions 0-63 and the softmax denominator on row 64
    (the baseline denominator trick). Head A -> bank cols 0:512, head B ->
    cols 512:1024 of one [128,1024] PSUM tile.
  - normalization: reciprocal of the denominator rows, GpSimd partition
    broadcast, two DVE multiplies -> attnN bf16.
  - O-projection in bf16; output drained f32 by DVE and DMA'd per 128-row
    tile.
  - Masked keys are compacted away on the host (their softmax weight is
    exactly 0 in the reference); keys padded to a multiple of 128.
  - Host blobs are laid out in DMA-consumption order so the first
    projection matmul starts ~2us after launch.
"""

import os
import numpy as np

B, S, D, H = 4, 2048, 512, 8
DH = D // H
NCORE = 8
SQ = S // 2  # queries per core
SCALE = 1.0 / float(np.sqrt(DH))  # 0.125

_BUILT = {}


def _chunks(total, step):
    out = []
    c0 = 0
    while c0 < total:
        out.append((c0, min(step, total - c0)))
        c0 += step
    return out


def _layout_k(s_pad):
    """DMA piece order for the K (and Q, with s_pad=SQ) blob."""
    pieces = [("wj0", 512)]
    rest_w = True
    for c0, cw in _chunks(s_pad, 1024):
        for h0, hw in _chunks(cw, 512):
            for dk in range(4):
                pieces.append((f"x_{c0}_{h0}_{dk}", hw))
            if rest_w:
                pieces.append(("wj1", 512))
                rest_w = False
        if ("wj2", 512) not in pieces:
            pieces.append(("wj2", 512))
            pieces.append(("wj3", 512))
    return pieces


def _layout_v(s_pad):
    nsk = s_pad // 128
    pieces = [(f"wv{dk}", 512) for dk in range(4)]
    for t0 in range(0, nsk, 2):
        tw = min(2, nsk - t0) * 128
        for dk in range(4):
            pieces.append((f"xv_{t0}_{dk}", tw))
    return pieces


def _offsets(pieces):
    off = {}
    c = 0
    for name, w in pieces:
        off[name] = c
        c += w
    return off, c


def build_bass(s_pad, has_bias=False):
    import concourse.bass as bass  # noqa: F401
    import concourse.mybir as mybir
    import concourse.tile as tile
    from concourse import bacc
    from contextlib import ExitStack

    f32 = mybir.dt.float32
    f32r = mybir.dt.float32r
    bf16 = mybir.dt.bfloat16
    EXP = mybir.ActivationFunctionType.Exp
    MUL = mybir.AluOpType.mult
    ADD = mybir.AluOpType.add

    nsk = s_pad // 128

    LK, WK = _offsets(_layout_k(s_pad))
    LQ, WQ = _offsets(_layout_k(SQ))
    LV, WV = _offsets(_layout_v(s_pad))

    nc = bacc.Bacc(
        "TRN2",
        target_bir_lowering=False,
        debug=False,
        enable_asserts=False,
        num_devices=NCORE,
    )

    d_bk = nc.dram_tensor("blob_k", [128, WK], bf16, kind="ExternalInput").ap()
    d_bq = nc.dram_tensor("blob_q", [128, WQ], bf16, kind="ExternalInput").ap()
    d_bv = nc.dram_tensor("blob_v", [128, WV], bf16, kind="ExternalInput").ap()
    d_bo = nc.dram_tensor("blob_o", [128, 2048 + 512], bf16, kind="ExternalInput").ap()
    d_mbe = nc.dram_tensor("mbe", [128, nsk], f32, kind="ExternalInput").ap()
    if has_bias:
        d_bkq = nc.dram_tensor("bkq_pp", [128, 8], f32, kind="ExternalInput").ap()
        d_bvb = nc.dram_tensor("bvb", [128, 512], f32, kind="ExternalInput").ap()
        d_bob = nc.dram_tensor("bob", [128, 512], f32, kind="ExternalInput").ap()
    d_out = nc.dram_tensor("out", [SQ, D], f32, kind="ExternalOutput").ap()
    debug = bool(os.environ.get("BASSK_DEBUG"))
    if debug:
        d_dbg_k = nc.dram_tensor("dbg_k", [128, s_pad], f32, kind="ExternalOutput").ap()
        d_dbg_q = nc.dram_tensor("dbg_q", [128, SQ], f32, kind="ExternalOutput").ap()
        d_dbg_v = nc.dram_tensor("dbg_v", [128, H * (DH + 1)], f32, kind="ExternalOutput").ap()
        d_dbg_w = nc.dram_tensor("dbg_w", [128, 1024], f32, kind="ExternalOutput").ap()
        d_dbg_av = nc.dram_tensor("dbg_av", [128, 1024], f32, kind="ExternalOutput").ap()

    with tile.TileContext(nc) as tc, ExitStack() as ctx, nc.allow_low_precision(
        "f32r attention path; bf16 projections"
    ):
        sb = ctx.enter_context(tc.tile_pool(name="sb", bufs=1))
        ps = ctx.enter_context(tc.tile_pool(name="ps", bufs=2, space="PSUM"))
        ps_av = ctx.enter_context(tc.tile_pool(name="psav", bufs=2, space="PSUM"))
        wx_p = ctx.enter_context(tc.tile_pool(name="wxp", bufs=21))
        rcp_p = ctx.enter_context(tc.tile_pool(name="rcpp", bufs=2))
        bcs_p = ctx.enter_context(tc.tile_pool(name="bcsp", bufs=2))
        osb_p = ctx.enter_context(tc.tile_pool(name="osbp", bufs=2))

        # persistent tiles
        kT = [sb.tile([128, s_pad], bf16, tag=f"kT{j}", name=f"kT{j}") for j in range(4)]
        qT = [sb.tile([128, SQ], bf16, tag=f"qT{j}", name=f"qT{j}") for j in range(4)]
        v = sb.tile([128, nsk, H, DH + 1], bf16, tag="v", name="v")
        attnN = [
            sb.tile([128, SQ], bf16, tag=f"attnN{pr}", name=f"attnN{pr}")
            for pr in range(4)
        ]
        mbe = sb.tile([128, nsk], f32, tag="mbe", name="mbe")
        nc.sync.dma_start(mbe[:], d_mbe[:])
        blo = sb.tile([128, 2048 + 512], bf16, tag="blo", name="blo")
        if has_bias:
            bkq = sb.tile([128, 8], f32, tag="bkq", name="bkq")
            nc.sync.dma_start(bkq[:], d_bkq[:])
            bvb = sb.tile([128, 512], f32, tag="bvb", name="bvb")
            nc.sync.dma_start(bvb[:], d_bvb[:])
            bob = sb.tile([128, 512], f32, tag="bob", name="bob")
            nc.sync.dma_start(bob[:], d_bob[:])

        # denominator ones column (drains overwrite the v parts)
        nc.gpsimd.memset(v[:].bitcast(mybir.dt.uint16), 0x3F80)

        with tc.tile_pool(name="inp", bufs=1) as inp:
            # Each dma_start fans out across all DMA queues, so a few large
            # transfers in consumption order beat many small ones (the SP
            # issue cost ~0.6us/DMA dominates otherwise).
            blk = inp.tile([128, WK], bf16, tag="blk", name="blk")
            blq = inp.tile([128, WQ], bf16, tag="blq", name="blq")
            blv = inp.tile([128, WV], bf16, tag="blv", name="blv")
            # K j0 data first, then Q j0, then the rests interleaved
            kcut = LK["wj1"]
            qcut = LQ["wj1"]
            for c0, cw in _chunks(kcut, 1024):
                nc.sync.dma_start(blk[:, c0 : c0 + cw], d_bk[:, c0 : c0 + cw])
            for c0, cw in _chunks(qcut, 1024):
                nc.gpsimd.dma_start(blq[:, c0 : c0 + cw], d_bq[:, c0 : c0 + cw])
            c0 = kcut
            while c0 < WK:
                c1 = min(c0 + 1536, WK)
                nc.sync.dma_start(blk[:, c0:c1], d_bk[:, c0:c1])
                c0 = c1
            c0 = qcut
            while c0 < WQ:
                c1 = min(c0 + 1536, WQ)
                nc.gpsimd.dma_start(blq[:, c0:c1], d_bq[:, c0:c1])
                c0 = c1
            for c0, cw in _chunks(WV, 2048):
                nc.sync.dma_start(blv[:, c0 : c0 + cw], d_bv[:, c0 : c0 + cw])
            nc.sync.dma_start(blo[:], d_bo[:])

            def proj_kq_j(bl, LX, xT, s_tot, bias_col, j):
                for c0, cw in _chunks(s_tot, 1024):
                    pt = ps.tile([128, 1024], f32, tag="lg", name="pp")
                    for h0, hw in _chunks(cw, 512):
                        for dk in range(4):
                            nc.tensor.matmul(
                                pt[:, h0 : h0 + hw],
                                lhsT=bl[:, LX[f"wj{j}"] + dk * 128 : LX[f"wj{j}"] + dk * 128 + 128],
                                rhs=bl[:, LX[f"x_{c0}_{h0}_{dk}"] : LX[f"x_{c0}_{h0}_{dk}"] + hw],
                                start=(dk == 0),
                                stop=(dk == 3),
                            )
                    if has_bias:
                        nc.vector.tensor_scalar_add(
                            xT[j][:, c0 : c0 + cw], pt[:, 0:cw], bkq[:, bias_col + j : bias_col + j + 1]
                        )
                    else:
                        nc.vector.tensor_copy(xT[j][:, c0 : c0 + cw], pt[:, 0:cw])

            def emit_logits_exp(qc, pr):
                # logits+exp only; PSUM from the (idle) av arena so the
                # projection pipeline in the lg arena is not paced by ACT
                q0 = qc * 512
                tiles = []
                for t in range(nsk):
                    wx = wx_p.tile([128, 1024], bf16, tag="wx", name="wx")
                    lg = ps_av.tile([128, 1024], f32, tag="av", name="elg")
                    nc.tensor.matmul(
                        lg[:, 0:512],
                        lhsT=kT[pr][0:64, t * 128 : (t + 1) * 128],
                        rhs=qT[pr][0:64, q0 : q0 + 512],
                        start=True,
                        stop=True,
                    )
                    nc.tensor.matmul(
                        lg[:, 512:1024],
                        lhsT=kT[pr][64:128, t * 128 : (t + 1) * 128],
                        rhs=qT[pr][64:128, q0 : q0 + 512],
                        start=True,
                        stop=True,
                    )
                    nc.scalar.activation(
                        wx[:], lg[:], EXP, bias=mbe[:, t : t + 1], scale=SCALE
                    )
                    tiles.append(wx)
                return tiles

            early = {}
            proj_kq_j(blk, LK, kT, s_pad, 0, 0)
            proj_kq_j(blq, LQ, qT, SQ, 4, 0)

            def v_proj_group(t0):
                tw = min(2, nsk - t0)
                pt = ps.tile([128, 1024], f32, tag="lg", name="vp")
                for jj in range(tw):
                    for dk in range(4):
                        o = LV[f"xv_{t0}_{dk}"] + jj * 128
                        nc.tensor.matmul(
                            pt[:, jj * 512 : (jj + 1) * 512],
                            lhsT=blv[:, o : o + 128],
                            rhs=blv[:, LV[f"wv{dk}"] : LV[f"wv{dk}"] + 512],
                            start=(dk == 0),
                            stop=(dk == 3),
                        )
                src = pt[:, 0 : tw * 512].rearrange("p (t h d) -> p t h d", t=tw, h=H)
                dst = v[:, t0 : t0 + tw, :, 0:DH]
                if has_bias:
                    nc.vector.scalar_tensor_tensor(
                        dst, src, 1.0,
                        bvb.rearrange("p (h d) -> p h d", h=H).broadcast(1, tw),
                        op0=MUL, op1=ADD,
                    )
                else:
                    nc.vector.tensor_copy(dst, src)

            def emit_one_logit_exp(qc, pr, t):
                q0 = qc * 512
                wx = wx_p.tile([128, 1024], bf16, tag="wx", name="wx")
                lg = ps_av.tile([128, 1024], f32, tag="av", name="elg")
                nc.tensor.matmul(
                    lg[:, 0:512],
                    lhsT=kT[pr][0:64, t * 128 : (t + 1) * 128],
                    rhs=qT[pr][0:64, q0 : q0 + 512],
                    start=True,
                    stop=True,
                )
                nc.tensor.matmul(
                    lg[:, 512:1024],
                    lhsT=kT[pr][64:128, t * 128 : (t + 1) * 128],
                    rhs=qT[pr][64:128, q0 : q0 + 512],
                    start=True,
                    stop=True,
                )
                nc.scalar.activation(
                    wx[:], lg[:], EXP, bias=mbe[:, t : t + 1], scale=SCALE
                )
                return wx

            # Remaining projection groups and V groups interleaved with
            # iteration (0,0)'s logits+exp: PE stays busy on projection
            # matmuls while ACT works through the early exps (the early-lg
            # arena is only 2 deep), and the exp stream starts right after
            # the j=0 projections land.
            fillers = []
            for j in (1, 2, 3):
                fillers.append(lambda j=j: proj_kq_j(blk, LK, kT, s_pad, 0, j))
                fillers.append(lambda j=j: proj_kq_j(blq, LQ, qT, SQ, 4, j))
            for t0 in range(0, nsk, 2):
                fillers.append(lambda t0=t0: v_proj_group(t0))
            e_tiles = []
            ei = iter(range(nsk))
            for fi, f in enumerate(fillers):
                f()
                if fi >= 1:
                    t = next(ei, None)
                    if t is not None:
                        e_tiles.append(emit_one_logit_exp(0, 0, t))
            for t in ei:
                e_tiles.append(emit_one_logit_exp(0, 0, t))
            early[(0, 0)] = e_tiles

        # ---- attention ----
        def finish(qc, pr, av):
            q0 = qc * 512
            if debug and qc == 0 and pr == 0:
                dav = sb.tile([128, 1024], f32, tag="dav", name="dav")
                nc.vector.tensor_copy(dav[:], av[:])
                nc.sync.dma_start(d_dbg_av[:], dav[:])
            dsA = rcp_p.tile([1, 512], f32, tag="dsA", name="dsA")
            dsB = rcp_p.tile([1, 512], f32, tag="dsB", name="dsB")
            nc.vector.tensor_copy(dsA[0:1, :], av[64:65, 0:512])
            nc.vector.tensor_copy(dsB[0:1, :], av[64:65, 512:1024])
            rfA = rcp_p.tile([1, 512], f32, tag="rfA", name="rfA")
            rfB = rcp_p.tile([1, 512], f32, tag="rfB", name="rfB")
            nc.vector.reciprocal_approx_fast(rfA[0:1, :], dsA[0:1, :])
            nc.vector.reciprocal_approx_fast(rfB[0:1, :], dsB[0:1, :])
            bcsA = bcs_p.tile([64, 512], f32, tag="bcsA", name="bcsA")
            bcsB = bcs_p.tile([64, 512], f32, tag="bcsB", name="bcsB")
            nc.gpsimd.partition_broadcast(bcsA[0:64, :], rfA[0:1, :], channels=64)
            nc.gpsimd.partition_broadcast(bcsB[0:64, :], rfB[0:1, :], channels=64)
            nc.vector.tensor_mul(
                attnN[pr][0:64, q0 : q0 + 512], av[0:64, 0:512], bcsA[0:64, :]
            )
            nc.vector.tensor_mul(
                attnN[pr][64:128, q0 : q0 + 512], av[0:64, 512:1024], bcsB[0:64, :]
            )

        def o_proj(qc):
            for qt2 in range(2):
                opst = ps.tile([128, 1024], f32, tag="lg", name="opst")
                for half in range(2):
                    qq = qc * 512 + (qt2 * 2 + half) * 128
                    for pr2 in range(4):
                        nc.tensor.matmul(
                            opst[:, half * 512 : (half + 1) * 512],
                            lhsT=attnN[pr2][:, qq : qq + 128],
                            rhs=blo[:, pr2 * 512 : (pr2 + 1) * 512],
                            start=(pr2 == 0),
                            stop=(pr2 == 3),
                        )
                for half in range(2):
                    osb = osb_p.tile([128, 512], f32, tag="osb", name="osb")
                    if has_bias:
                        nc.vector.scalar_tensor_tensor(
                            osb[:], opst[:, half * 512 : (half + 1) * 512],
                            1.0, bob[:], op0=MUL, op1=ADD,
                        )
                    else:
                        nc.vector.tensor_copy(osb[:], opst[:, half * 512 : (half + 1) * 512])
                    qq = qc * 512 + (qt2 * 2 + half) * 128
                    nc.sync.dma_start(d_out[qq : qq + 128, :], osb[:])

        iters = [(qc, pr) for qc in range(SQ // 512) for pr in range(4)]
        n_it = len(iters)
        wx_prev = None
        av_of = {}
        for s in range(n_it + 1):
            if s >= 2:
                qc2, pr2 = iters[s - 2]
                finish(qc2, pr2, av_of.pop((qc2, pr2)))
                if pr2 == 3:
                    o_proj(qc2)
            cur = iters[s] if s < n_it else None
            prv = iters[s - 1] if s >= 1 else None
            av = None
            if prv is not None:
                av = ps_av.tile([128, 1024], f32, tag="av", name="av")
                hA, hB = 2 * prv[1], 2 * prv[1] + 1
            wx_l = []
            pre = early.pop(cur, None) if cur is not None else None
            for t in range(nsk):
                if prv is not None:
                    wx = wx_prev[t]
                    last = t == nsk - 1
                    nc.tensor.matmul(
                        av[0:65, 0:512],
                        lhsT=v[:, t, hA, :],
                        rhs=wx[:, 0:512],
                        start=(t == 0),
                        stop=last,
                    )
                    nc.tensor.matmul(
                        av[0:65, 512:1024],
                        lhsT=v[:, t, hB, :],
                        rhs=wx[:, 512:1024],
                        start=(t == 0),
                        stop=last,
                    )
                if cur is not None:
                    if pre is not None:
                        wx_l.append(pre[t])
                    else:
                        qc, pr = cur
                        q0 = qc * 512
                        wx = wx_p.tile([128, 1024], bf16, tag="wx", name="wx")
                        lg = ps.tile([128, 1024], f32, tag="lg", name="lg")
                        nc.tensor.matmul(
                            lg[:, 0:512],
                            lhsT=kT[pr][0:64, t * 128 : (t + 1) * 128],
                            rhs=qT[pr][0:64, q0 : q0 + 512],
                            start=True,
                            stop=True,
                        )
                        nc.tensor.matmul(
                            lg[:, 512:1024],
                            lhsT=kT[pr][64:128, t * 128 : (t + 1) * 128],
                            rhs=qT[pr][64:128, q0 : q0 + 512],
                            start=True,
                            stop=True,
                        )
                        nc.scalar.activation(
                            wx[:], lg[:], EXP, bias=mbe[:, t : t + 1], scale=SCALE
                        )
                        wx_l.append(wx)
            if prv is not None:
                av_of[prv] = av
            wx_prev = wx_l if cur is not None else None
        qcl, prl = iters[-1]
        finish(qcl, prl, av_of.pop((qcl, prl)))
        o_proj(qcl)

    nc.compile()
    return nc


def _prep_inputs(query, key, value, mask, wq_w, wq_b, wk_w, wk_b, wv_w, wv_b, wo_w, wo_b):
    import ml_dtypes

    bf = ml_dtypes.bfloat16
    f = lambda a: np.ascontiguousarray(np.asarray(a, dtype=np.float32))
    query, key, value = f(query), f(key), f(value)
    wq_w, wk_w, wv_w, wo_w = f(wq_w), f(wk_w), f(wv_w), f(wo_w)
    wq_b, wk_b, wv_b, wo_b = f(wq_b), f(wk_b), f(wv_b), f(wo_b)
    mask = np.asarray(mask)

    has_bias = bool(np.any(wq_b) or np.any(wk_b) or np.any(wv_b) or np.any(wo_b))

    keeps = [np.flatnonzero(mask[b] == 0) for b in range(B)]
    cnts = [len(k) for k in keeps]
    assert min(cnts) > 0, "all-masked batch not supported"
    s_pad = max(128, ((max(cnts) + 127) // 128) * 128)
    nsk = s_pad // 128

    def build_blob(pieces, off, total, w_full, xT):
        blob = np.empty((128, total), np.float32)
        for name, w in pieces:
            o = off[name]
            if name.startswith("wj"):
                j = int(name[2])
                for dk in range(4):
                    blob[:, o + dk * 128 : o + (dk + 1) * 128] = w_full[
                        dk * 128 : (dk + 1) * 128, j * 128 : (j + 1) * 128
                    ]
            else:
                _, c0, h0, dk = name.split("_")
                c0, h0, dk = int(c0), int(h0), int(dk)
                blob[:, o : o + w] = xT[dk * 128 : (dk + 1) * 128, c0 + h0 : c0 + h0 + w]
        return np.ascontiguousarray(blob.astype(bf))

    def build_blob_v(pieces, off, total, w_full, xT):
        blob = np.empty((128, total), np.float32)
        for name, w in pieces:
            o = off[name]
            if name.startswith("wv"):
                dk = int(name[2])
                blob[:, o : o + 512] = w_full[dk * 128 : (dk + 1) * 128, :]
            else:
                _, t0, dk = name.split("_")
                t0, dk = int(t0), int(dk)
                blob[:, o : o + w] = xT[
                    dk * 128 : (dk + 1) * 128, t0 * 128 : t0 * 128 + w
                ]
        return np.ascontiguousarray(blob.astype(bf))

    LKp = _layout_k(s_pad)
    LK, WK = _offsets(LKp)
    LQp = _layout_k(SQ)
    LQ, WQ = _offsets(LQp)
    LVp = _layout_v(s_pad)
    LV, WV = _offsets(LVp)

    blob_o = np.concatenate(
        [wo_w.reshape(4, 128, 512).transpose(1, 0, 2).reshape(128, 2048),
         np.broadcast_to(wo_b.reshape(1, D), (128, D))],
        axis=1,
    )
    blob_o = np.ascontiguousarray(blob_o.astype(bf))

    common = dict(blob_o=blob_o)
    if has_bias:
        bkq = np.concatenate([wk_b.reshape(4, 128).T, wq_b.reshape(4, 128).T], axis=1)
        common.update(
            bkq_pp=np.ascontiguousarray(bkq),
            bvb=np.ascontiguousarray(np.broadcast_to(wv_b.reshape(1, D), (128, D))),
            bob=np.ascontiguousarray(np.broadcast_to(wo_b.reshape(1, D), (128, D))),
        )

    in_maps = []
    for b in range(B):
        kc = np.zeros((s_pad, D), np.float32)
        kc[: cnts[b]] = key[b][keeps[b]]
        vc = np.zeros((s_pad, D), np.float32)
        vc[: cnts[b]] = value[b][keeps[b]]
        blob_k = build_blob(LKp, LK, WK, wk_w, np.ascontiguousarray(kc.T))
        blob_v = build_blob_v(LVp, LV, WV, wv_w, np.ascontiguousarray(vc.T))
        mbef = np.full(s_pad, -1e9, np.float32)
        mbef[: cnts[b]] = 0.0
        mbe = np.ascontiguousarray(mbef.reshape(nsk, 128).T)
        for qh in range(2):
            xq = np.ascontiguousarray(query[b, qh * SQ : (qh + 1) * SQ, :].T)
            blob_q = build_blob(LQp, LQ, WQ, wq_w, xq)
            in_maps.append(
                dict(blob_k=blob_k, blob_q=blob_q, blob_v=blob_v, mbe=mbe, **common)
            )
    return s_pad, has_bias, in_maps


def kernel(**inputs):
    from concourse import bass_utils

    s_pad, has_bias, in_maps = _prep_inputs(**inputs)
    bkey = (s_pad, has_bias)
    if bkey not in _BUILT:
        _BUILT[bkey] = build_bass(s_pad, has_bias=has_bias)
    nc = _BUILT[bkey]
    kw = {}
    if os.environ.get("BASSK_TRACE"):
        kw = dict(trace=True, stitch_traces=False)
    res = bass_utils.run_bass_kernel_spmd(nc, in_maps, core_ids=list(range(NCORE)), **kw)
    out = np.empty((B, S, D), np.float32)
    for c in range(NCORE):
        b, qh = c // 2, c % 2
        out[b, qh * SQ : (qh + 1) * SQ, :] = res.results[c]["out"]
    kernel.last_result = res
    return out


# revision 26
# speedup vs baseline: 1.0288x; 1.0288x over previous
"""Multi-head attention (B=4, S=2048, D=512, H=8) on 8 trn2 NeuronCores.

Sharding: core c = (batch b = c//2, query-half qh = c%2). Each core computes
the full attention output for 1024 query rows of one batch element.

Device-side scheme:
  - K/Q/V projections in bf16 at full PE rate (FWL weight loads); kT/qT/v
    drained to SBUF as float32r so logits and AV matmuls run on the
    full-rate fp32 PE path with ~fp22 accuracy (f32r costs the same PE
    streaming time as bf16 at N=512, and the same LDWEIGHTS time for these
    shapes since K=64 / M=65 never qualify for fast-weight-load anyway).
  - Logits transposed (lg^T[s_k, q]), two heads per pass as concurrent
    K=64 row-tiles (partitions 0-63 / 64-127).
  - exp on ACT (f32 PSUM -> f32r SBUF) with a per-key-partition mask bias;
    no max-subtraction (logits*scale ~ N(0,1); exp is safe in fp32) and
    masked/padded keys get bias -1e9 -> exp exactly 0.
  - AV: per key tile, lhsT = [v_h | ones] (M=65) so PSUM gets the attention
    numerator on partitions 0-63 and the softmax denominator on row 64
    (the baseline denominator trick). Head A -> bank cols 0:512, head B ->
    cols 512:1024 of one [128,1024] PSUM tile.
  - normalization: reciprocal of the denominator rows, GpSimd partition
    broadcast, two DVE multiplies -> attnN bf16.
  - O-projection in bf16; output drained f32 by DVE and DMA'd per 128-row
    tile.
  - Masked keys are compacted away on the host (their softmax weight is
    exactly 0 in the reference); keys padded to a multiple of 128.
  - Host blobs are laid out in DMA-consumption order so the first
    projection matmul starts ~2us after launch.
"""

import os
import numpy as np

B, S, D, H = 4, 2048, 512, 8
DH = D // H
NCORE = 8
SQ = S // 2  # queries per core
SCALE = 1.0 / float(np.sqrt(DH))  # 0.125

_BUILT = {}


def _chunks(total, step):
    out = []
    c0 = 0
    while c0 < total:
        out.append((c0, min(step, total - c0)))
        c0 += step
    return out


def _layout_k(s_pad):
    """DMA piece order for the K (and Q, with s_pad=SQ) blob."""
    pieces = [("wj0", 512)]
    rest_w = True
    for c0, cw in _chunks(s_pad, 1024):
        for h0, hw in _chunks(cw, 512):
            for dk in range(4):
                pieces.append((f"x_{c0}_{h0}_{dk}", hw))
            if rest_w:
                pieces.append(("wj1", 512))
                rest_w = False
        if ("wj2", 512) not in pieces:
            pieces.append(("wj2", 512))
            pieces.append(("wj3", 512))
    return pieces


def _layout_v(s_pad):
    nsk = s_pad // 128
    pieces = [(f"wv{dk}", 512) for dk in range(4)]
    for t0 in range(0, nsk, 2):
        tw = min(2, nsk - t0) * 128
        for dk in range(4):
            pieces.append((f"xv_{t0}_{dk}", tw))
    return pieces


def _offsets(pieces):
    off = {}
    c = 0
    for name, w in pieces:
        off[name] = c
        c += w
    return off, c


def build_bass(s_pad, has_bias=False):
    import concourse.bass as bass  # noqa: F401
    import concourse.mybir as mybir
    import concourse.tile as tile
    from concourse import bacc
    from contextlib import ExitStack

    f32 = mybir.dt.float32
    f32r = mybir.dt.float32r
    bf16 = mybir.dt.bfloat16
    EXP = mybir.ActivationFunctionType.Exp
    MUL = mybir.AluOpType.mult
    ADD = mybir.AluOpType.add

    nsk = s_pad // 128

    LK, WK = _offsets(_layout_k(s_pad))
    LQ, WQ = _offsets(_layout_k(SQ))
    LV, WV = _offsets(_layout_v(s_pad))

    nc = bacc.Bacc(
        "TRN2",
        target_bir_lowering=False,
        debug=False,
        enable_asserts=False,
        num_devices=NCORE,
    )

    d_bk = nc.dram_tensor("blob_k", [128, WK], bf16, kind="ExternalInput").ap()
    d_bq = nc.dram_tensor("blob_q", [128, WQ], bf16, kind="ExternalInput").ap()
    d_bv = nc.dram_tensor("blob_v", [128, WV], bf16, kind="ExternalInput").ap()
    d_bo = nc.dram_tensor("blob_o", [128, 2048 + 512], bf16, kind="ExternalInput").ap()
    d_mbe = nc.dram_tensor("mbe", [128, nsk], f32, kind="ExternalInput").ap()
    if has_bias:
        d_bkq = nc.dram_tensor("bkq_pp", [128, 8], f32, kind="ExternalInput").ap()
        d_bvb = nc.dram_tensor("bvb", [128, 512], f32, kind="ExternalInput").ap()
        d_bob = nc.dram_tensor("bob", [128, 512], f32, kind="ExternalInput").ap()
    d_out = nc.dram_tensor("out", [SQ, D], f32, kind="ExternalOutput").ap()
    debug = bool(os.environ.get("BASSK_DEBUG"))
    if debug:
        d_dbg_k = nc.dram_tensor("dbg_k", [128, s_pad], f32, kind="ExternalOutput").ap()
        d_dbg_q = nc.dram_tensor("dbg_q", [128, SQ], f32, kind="ExternalOutput").ap()
        d_dbg_v = nc.dram_tensor("dbg_v", [128, H * (DH + 1)], f32, kind="ExternalOutput").ap()
        d_dbg_w = nc.dram_tensor("dbg_w", [128, 1024], f32, kind="ExternalOutput").ap()
        d_dbg_av = nc.dram_tensor("dbg_av", [128, 1024], f32, kind="ExternalOutput").ap()

    with tile.TileContext(nc) as tc, ExitStack() as ctx, nc.allow_low_precision(
        "f32r attention path; bf16 projections"
    ):
        sb = ctx.enter_context(tc.tile_pool(name="sb", bufs=1))
        ps = ctx.enter_context(tc.tile_pool(name="ps", bufs=2, space="PSUM"))
        ps_av = ctx.enter_context(tc.tile_pool(name="psav", bufs=2, space="PSUM"))
        wx_p = ctx.enter_context(tc.tile_pool(name="wxp", bufs=21))
        rcp_p = ctx.enter_context(tc.tile_pool(name="rcpp", bufs=2))
        bcs_p = ctx.enter_context(tc.tile_pool(name="bcsp", bufs=2))
        osb_p = ctx.enter_context(tc.tile_pool(name="osbp", bufs=2))

        # persistent tiles
        kT = [sb.tile([128, s_pad], bf16, tag=f"kT{j}", name=f"kT{j}") for j in range(4)]
        qT = [sb.tile([128, SQ], bf16, tag=f"qT{j}", name=f"qT{j}") for j in range(4)]
        v = sb.tile([128, nsk, H, DH + 1], bf16, tag="v", name="v")
        attnN = [
            sb.tile([128, SQ], bf16, tag=f"attnN{pr}", name=f"attnN{pr}")
            for pr in range(4)
        ]
        mbe = sb.tile([128, nsk], f32, tag="mbe", name="mbe")
        nc.sync.dma_start(mbe[:], d_mbe[:])
        blo = sb.tile([128, 2048 + 512], bf16, tag="blo", name="blo")
        if has_bias:
            bkq = sb.tile([128, 8], f32, tag="bkq", name="bkq")
            nc.sync.dma_start(bkq[:], d_bkq[:])
            bvb = sb.tile([128, 512], f32, tag="bvb", name="bvb")
            nc.sync.dma_start(bvb[:], d_bvb[:])
            bob = sb.tile([128, 512], f32, tag="bob", name="bob")
            nc.sync.dma_start(bob[:], d_bob[:])

        # denominator ones column (drains overwrite the v parts)
        nc.gpsimd.memset(v[:].bitcast(mybir.dt.uint16), 0x3F80)

        with tc.tile_pool(name="inp", bufs=1) as inp:
            # Each dma_start fans out across all DMA queues, so a few large
            # transfers in consumption order beat many small ones (the SP
            # issue cost ~0.6us/DMA dominates otherwise).
            blk = inp.tile([128, WK], bf16, tag="blk", name="blk")
            blq = inp.tile([128, WQ], bf16, tag="blq", name="blq")
            blv = inp.tile([128, WV], bf16, tag="blv", name="blv")
            # K j0 data first, then Q j0, then the rests interleaved
            kcut = LK["wj1"]
            qcut = LQ["wj1"]
            for c0, cw in _chunks(kcut, 1024):
                nc.sync.dma_start(blk[:, c0 : c0 + cw], d_bk[:, c0 : c0 + cw])
            for c0, cw in _chunks(qcut, 1024):
                nc.gpsimd.dma_start(blq[:, c0 : c0 + cw], d_bq[:, c0 : c0 + cw])
            c0 = kcut
            while c0 < WK:
                c1 = min(c0 + 1536, WK)
                nc.sync.dma_start(blk[:, c0:c1], d_bk[:, c0:c1])
                c0 = c1
            c0 = qcut
            while c0 < WQ:
                c1 = min(c0 + 1536, WQ)
                nc.gpsimd.dma_start(blq[:, c0:c1], d_bq[:, c0:c1])
                c0 = c1
            for c0, cw in _chunks(WV, 2048):
                nc.sync.dma_start(blv[:, c0 : c0 + cw], d_bv[:, c0 : c0 + cw])
            nc.sync.dma_start(blo[:], d_bo[:])

            def proj_kq_j(bl, LX, xT, s_tot, bias_col, j):
                for c0, cw in _chunks(s_tot, 1024):
                    pt = ps.tile([128, 1024], f32, tag="lg", name="pp")
                    for h0, hw in _chunks(cw, 512):
                        for dk in range(4):
                            nc.tensor.matmul(
                                pt[:, h0 : h0 + hw],
                                lhsT=bl[:, LX[f"wj{j}"] + dk * 128 : LX[f"wj{j}"] + dk * 128 + 128],
                                rhs=bl[:, LX[f"x_{c0}_{h0}_{dk}"] : LX[f"x_{c0}_{h0}_{dk}"] + hw],
                                start=(dk == 0),
                                stop=(dk == 3),
                            )
                    if has_bias:
                        nc.vector.tensor_scalar_add(
                            xT[j][:, c0 : c0 + cw], pt[:, 0:cw], bkq[:, bias_col + j : bias_col + j + 1]
                        )
                    else:
                        nc.scalar.copy(xT[j][:, c0 : c0 + cw], pt[:, 0:cw])

            def emit_logits_exp(qc, pr):
                # logits+exp only; PSUM from the (idle) av arena so the
                # projection pipeline in the lg arena is not paced by ACT
                q0 = qc * 512
                tiles = []
                for t in range(nsk):
                    wx = wx_p.tile([128, 1024], bf16, tag="wx", name="wx")
                    lg = ps_av.tile([128, 1024], f32, tag="av", name="elg")
                    nc.tensor.matmul(
                        lg[:, 0:512],
                        lhsT=kT[pr][0:64, t * 128 : (t + 1) * 128],
                        rhs=qT[pr][0:64, q0 : q0 + 512],
                        start=True,
                        stop=True,
                    )
                    nc.tensor.matmul(
                        lg[:, 512:1024],
                        lhsT=kT[pr][64:128, t * 128 : (t + 1) * 128],
                        rhs=qT[pr][64:128, q0 : q0 + 512],
                        start=True,
                        stop=True,
                    )
                    nc.scalar.activation(
                        wx[:], lg[:], EXP, bias=mbe[:, t : t + 1], scale=SCALE
                    )
                    tiles.append(wx)
                return tiles

            early = {}
            for j in range(4):
                proj_kq_j(blk, LK, kT, s_pad, 0, j)
            for j in range(4):
                proj_kq_j(blq, LQ, qT, SQ, 4, j)

            def v_proj_group(t0):
                tw = min(2, nsk - t0)
                pt = ps.tile([128, 1024], f32, tag="lg", name="vp")
                for jj in range(tw):
                    for dk in range(4):
                        o = LV[f"xv_{t0}_{dk}"] + jj * 128
                        nc.tensor.matmul(
                            pt[:, jj * 512 : (jj + 1) * 512],
                            lhsT=blv[:, o : o + 128],
                            rhs=blv[:, LV[f"wv{dk}"] : LV[f"wv{dk}"] + 512],
                            start=(dk == 0),
                            stop=(dk == 3),
                        )
                src = pt[:, 0 : tw * 512].rearrange("p (t h d) -> p t h d", t=tw, h=H)
                dst = v[:, t0 : t0 + tw, :, 0:DH]
                if has_bias:
                    nc.vector.scalar_tensor_tensor(
                        dst, src, 1.0,
                        bvb.rearrange("p (h d) -> p h d", h=H).broadcast(1, tw),
                        op0=MUL, op1=ADD,
                    )
                else:
                    nc.vector.tensor_copy(dst, src)

            def emit_one_logit_exp(qc, pr, t):
                q0 = qc * 512
                wx = wx_p.tile([128, 1024], bf16, tag="wx", name="wx")
                lg = ps_av.tile([128, 1024], f32, tag="av", name="elg")
                nc.tensor.matmul(
                    lg[:, 0:512],
                    lhsT=kT[pr][0:64, t * 128 : (t + 1) * 128],
                    rhs=qT[pr][0:64, q0 : q0 + 512],
                    start=True,
                    stop=True,
                )
                nc.tensor.matmul(
                    lg[:, 512:1024],
                    lhsT=kT[pr][64:128, t * 128 : (t + 1) * 128],
                    rhs=qT[pr][64:128, q0 : q0 + 512],
                    start=True,
                    stop=True,
                )
                nc.scalar.activation(
                    wx[:], lg[:], EXP, bias=mbe[:, t : t + 1], scale=SCALE
                )
                return wx

            # V-projection interleaved with iteration (0,0)'s logits+exp:
            # PE stays busy on V groups while ACT works through the early
            # exps (the early-lg arena is only 2 deep)
            e_tiles = []
            ei = iter(range(nsk))
            for t0 in range(0, nsk, 2):
                v_proj_group(t0)
                for _ in range(2):
                    t = next(ei, None)
                    if t is not None:
                        e_tiles.append(emit_one_logit_exp(0, 0, t))
            for t in ei:
                e_tiles.append(emit_one_logit_exp(0, 0, t))
            early[(0, 0)] = e_tiles

        # ---- attention ----
        def finish(qc, pr, av):
            q0 = qc * 512
            if debug and qc == 0 and pr == 0:
                dav = sb.tile([128, 1024], f32, tag="dav", name="dav")
                nc.vector.tensor_copy(dav[:], av[:])
                nc.sync.dma_start(d_dbg_av[:], dav[:])
            dsA = rcp_p.tile([1, 512], f32, tag="dsA", name="dsA")
            dsB = rcp_p.tile([1, 512], f32, tag="dsB", name="dsB")
            nc.vector.tensor_copy(dsA[0:1, :], av[64:65, 0:512])
            nc.vector.tensor_copy(dsB[0:1, :], av[64:65, 512:1024])
            rfA = rcp_p.tile([1, 512], f32, tag="rfA", name="rfA")
            rfB = rcp_p.tile([1, 512], f32, tag="rfB", name="rfB")
            nc.vector.reciprocal_approx_fast(rfA[0:1, :], dsA[0:1, :])
            nc.vector.reciprocal_approx_fast(rfB[0:1, :], dsB[0:1, :])
            bcsA = bcs_p.tile([64, 512], f32, tag="bcsA", name="bcsA")
            bcsB = bcs_p.tile([64, 512], f32, tag="bcsB", name="bcsB")
            nc.gpsimd.partition_broadcast(bcsA[0:64, :], rfA[0:1, :], channels=64)
            nc.gpsimd.partition_broadcast(bcsB[0:64, :], rfB[0:1, :], channels=64)
            nc.vector.tensor_mul(
                attnN[pr][0:64, q0 : q0 + 512], av[0:64, 0:512], bcsA[0:64, :]
            )
            nc.vector.tensor_mul(
                attnN[pr][64:128, q0 : q0 + 512], av[0:64, 512:1024], bcsB[0:64, :]
            )

        def o_proj(qc):
            for qt2 in range(2):
                opst = ps.tile([128, 1024], f32, tag="lg", name="opst")
                for half in range(2):
                    qq = qc * 512 + (qt2 * 2 + half) * 128
                    for pr2 in range(4):
                        nc.tensor.matmul(
                            opst[:, half * 512 : (half + 1) * 512],
                            lhsT=attnN[pr2][:, qq : qq + 128],
                            rhs=blo[:, pr2 * 512 : (pr2 + 1) * 512],
                            start=(pr2 == 0),
                            stop=(pr2 == 3),
                        )
                for half in range(2):
                    osb = osb_p.tile([128, 512], f32, tag="osb", name="osb")
                    if has_bias:
                        nc.vector.scalar_tensor_tensor(
                            osb[:], opst[:, half * 512 : (half + 1) * 512],
                            1.0, bob[:], op0=MUL, op1=ADD,
                        )
                    else:
                        nc.vector.tensor_copy(osb[:], opst[:, half * 512 : (half + 1) * 512])
                    qq = qc * 512 + (qt2 * 2 + half) * 128
                    nc.sync.dma_start(d_out[qq : qq + 128, :], osb[:])

        iters = [(qc, pr) for qc in range(SQ // 512) for pr in range(4)]
        n_it = len(iters)
        wx_prev = None
        av_of = {}
        for s in range(n_it + 1):
            if s >= 2:
                qc2, pr2 = iters[s - 2]
                finish(qc2, pr2, av_of.pop((qc2, pr2)))
                if pr2 == 3:
                    o_proj(qc2)
            cur = iters[s] if s < n_it else None
            prv = iters[s - 1] if s >= 1 else None
            av = None
            if prv is not None:
                av = ps_av.tile([128, 1024], f32, tag="av", name="av")
                hA, hB = 2 * prv[1], 2 * prv[1] + 1
            wx_l = []
            pre = early.pop(cur, None) if cur is not None else None
            for t in range(nsk):
                if prv is not None:
                    wx = wx_prev[t]
                    last = t == nsk - 1
                    nc.tensor.matmul(
                        av[0:65, 0:512],
                        lhsT=v[:, t, hA, :],
                        rhs=wx[:, 0:512],
                        start=(t == 0),
                        stop=last,
                    )
                    nc.tensor.matmul(
                        av[0:65, 512:1024],
                        lhsT=v[:, t, hB, :],
                        rhs=wx[:, 512:1024],
                        start=(t == 0),
                        stop=last,
                    )
                if cur is not None:
                    if pre is not None:
                        wx_l.append(pre[t])
                    else:
                        qc, pr = cur
                        q0 = qc * 512
                        wx = wx_p.tile([128, 1024], bf16, tag="wx", name="wx")
                        lg = ps.tile([128, 1024], f32, tag="lg", name="lg")
                        nc.tensor.matmul(
                            lg[:, 0:512],
                            lhsT=kT[pr][0:64, t * 128 : (t + 1) * 128],
                            rhs=qT[pr][0:64, q0 : q0 + 512],
                            start=True,
                            stop=True,
                        )
                        nc.tensor.matmul(
                            lg[:, 512:1024],
                            lhsT=kT[pr][64:128, t * 128 : (t + 1) * 128],
                            rhs=qT[pr][64:128, q0 : q0 + 512],
                            start=True,
                            stop=True,
                        )
                        nc.scalar.activation(
                            wx[:], lg[:], EXP, bias=mbe[:, t : t + 1], scale=SCALE
                        )
                        wx_l.append(wx)
            if prv is not None:
                av_of[prv] = av
            wx_prev = wx_l if cur is not None else None
        qcl, prl = iters[-1]
        finish(qcl, prl, av_of.pop((qcl, prl)))
        o_proj(qcl)

    nc.compile()
    return nc


def _prep_inputs(query, key, value, mask, wq_w, wq_b, wk_w, wk_b, wv_w, wv_b, wo_w, wo_b):
    import ml_dtypes

    bf = ml_dtypes.bfloat16
    f = lambda a: np.ascontiguousarray(np.asarray(a, dtype=np.float32))
    query, key, value = f(query), f(key), f(value)
    wq_w, wk_w, wv_w, wo_w = f(wq_w), f(wk_w), f(wv_w), f(wo_w)
    wq_b, wk_b, wv_b, wo_b = f(wq_b), f(wk_b), f(wv_b), f(wo_b)
    mask = np.asarray(mask)

    has_bias = bool(np.any(wq_b) or np.any(wk_b) or np.any(wv_b) or np.any(wo_b))

    keeps = [np.flatnonzero(mask[b] == 0) for b in range(B)]
    cnts = [len(k) for k in keeps]
    assert min(cnts) > 0, "all-masked batch not supported"
    s_pad = max(128, ((max(cnts) + 127) // 128) * 128)
    nsk = s_pad // 128

    def build_blob(pieces, off, total, w_full, xT):
        blob = np.empty((128, total), np.float32)
        for name, w in pieces:
            o = off[name]
            if name.startswith("wj"):
                j = int(name[2])
                for dk in range(4):
                    blob[:, o + dk * 128 : o + (dk + 1) * 128] = w_full[
                        dk * 128 : (dk + 1) * 128, j * 128 : (j + 1) * 128
                    ]
            else:
                _, c0, h0, dk = name.split("_")
                c0, h0, dk = int(c0), int(h0), int(dk)
                blob[:, o : o + w] = xT[dk * 128 : (dk + 1) * 128, c0 + h0 : c0 + h0 + w]
        return np.ascontiguousarray(blob.astype(bf))

    def build_blob_v(pieces, off, total, w_full, xT):
        blob = np.empty((128, total), np.float32)
        for name, w in pieces:
            o = off[name]
            if name.startswith("wv"):
                dk = int(name[2])
                blob[:, o : o + 512] = w_full[dk * 128 : (dk + 1) * 128, :]
            else:
                _, t0, dk = name.split("_")
                t0, dk = int(t0), int(dk)
                blob[:, o : o + w] = xT[
                    dk * 128 : (dk + 1) * 128, t0 * 128 : t0 * 128 + w
                ]
        return np.ascontiguousarray(blob.astype(bf))

    LKp = _layout_k(s_pad)
    LK, WK = _offsets(LKp)
    LQp = _layout_k(SQ)
    LQ, WQ = _offsets(LQp)
    LVp = _layout_v(s_pad)
    LV, WV = _offsets(LVp)

    blob_o = np.concatenate(
        [wo_w.reshape(4, 128, 512).transpose(1, 0, 2).reshape(128, 2048),
         np.broadcast_to(wo_b.reshape(1, D), (128, D))],
        axis=1,
    )
    blob_o = np.ascontiguousarray(blob_o.astype(bf))

    common = dict(blob_o=blob_o)
    if has_bias:
        bkq = np.concatenate([wk_b.reshape(4, 128).T, wq_b.reshape(4, 128).T], axis=1)
        common.update(
            bkq_pp=np.ascontiguousarray(bkq),
            bvb=np.ascontiguousarray(np.broadcast_to(wv_b.reshape(1, D), (128, D))),
            bob=np.ascontiguousarray(np.broadcast_to(wo_b.reshape(1, D), (128, D))),
        )

    in_maps = []
    for b in range(B):
        kc = np.zeros((s_pad, D), np.float32)
        kc[: cnts[b]] = key[b][keeps[b]]
        vc = np.zeros((s_pad, D), np.float32)
        vc[: cnts[b]] = value[b][keeps[b]]
        blob_k = build_blob(LKp, LK, WK, wk_w, np.ascontiguousarray(kc.T))
        blob_v = build_blob_v(LVp, LV, WV, wv_w, np.ascontiguousarray(vc.T))
        mbef = np.full(s_pad, -1e9, np.float32)
        mbef[: cnts[b]] = 0.0
        mbe = np.ascontiguousarray(mbef.reshape(nsk, 128).T)
        for qh in range(2):
            xq = np.ascontiguousarray(query[b, qh * SQ : (qh + 1) * SQ, :].T)
            blob_q = build_blob(LQp, LQ, WQ, wq_w, xq)
            in_maps.append(
                dict(blob_k=blob_k, blob_q=blob_q, blob_v=blob_v, mbe=mbe, **common)
            )
    return s_pad, has_bias, in_maps


def kernel(**inputs):
    from concourse import bass_utils

    s_pad, has_bias, in_maps = _prep_inputs(**inputs)
    bkey = (s_pad, has_bias)
    if bkey not in _BUILT:
        _BUILT[bkey] = build_bass(s_pad, has_bias=has_bias)
    nc = _BUILT[bkey]
    kw = {}
    if os.environ.get("BASSK_TRACE"):
        kw = dict(trace=True, stitch_traces=False)
    res = bass_utils.run_bass_kernel_spmd(nc, in_maps, core_ids=list(range(NCORE)), **kw)
    out = np.empty((B, S, D), np.float32)
    for c in range(NCORE):
        b, qh = c // 2, c % 2
        out[b, qh * SQ : (qh + 1) * SQ, :] = res.results[c]["out"]
    kernel.last_result = res
    return out
